# revision 5
# baseline (speedup 1.0000x reference)
"""CommNet (B=4096, A=50, DIN=128, H=256, DOUT=64, K=2) on 8 TRN2 NeuronCores.

Data-parallel over the batch axis: 512 examples (25600 agent-tokens) per core,
weights replicated. On-chip layout is feature-major ([feature, token]) so every
layer's contraction dim sits on SBUF partitions; the host pre-transposes each
x shard once (numpy) so no on-chip transposes are needed.

Per comm step the concat [h, c] @ W is split as h @ W_top + c @ W_bot with the
1/50 agent-mean folded into W_bot on the host; the per-example c @ W_bot result
is broadcast back over agents with a stride-0 access pattern in the DVE add.

Engine budget per token per core (warm): PE ~12.6 cyc (f32r matmuls at
1 cyc/row), ACT 6 (tanh), DVE ~5.2 (broadcast adds + decoder bias copy),
POOL 4 (agent-sum reduces).
"""

import numpy as np

import concourse.bacc as bacc
import concourse.bass as bass
import concourse.tile as tile
from concourse import mybir
from concourse.bass_utils import run_bass_kernel_spmd

N_CORES = 8
B, A, DIN, H, DOUT, K = 4096, 50, 128, 256, 64, 2
BS = B // N_CORES          # examples per core
TOK = BS * A               # tokens per core
ST_EX = 64                 # examples per supertile
ST = ST_EX * A             # 3200 tokens per supertile
SUB_EX = 8                 # examples per matmul sub-tile
SUB = SUB_EX * A           # 400 tokens (matmul moving dim, <=512 for f32)
NSUB = ST // SUB

F32 = mybir.dt.float32
F32R = mybir.dt.float32r
Tanh = mybir.ActivationFunctionType.Tanh


def _r(ap):
    """f32r view: full-rate (1 cyc/row at N>=256) fp32 path through the PE."""
    return ap.bitcast(F32R)


def build_nc(n_supertiles=BS // ST_EX):
    tok = n_supertiles * ST
    nc = bacc.Bacc(
        "TRN2",
        target_bir_lowering=False,
        debug=False,
        enable_asserts=True,
        num_devices=N_CORES,
    )
    xT = nc.dram_tensor("xT", [DIN, tok], F32, kind="ExternalInput")
    w_enc = nc.dram_tensor("w_enc", [DIN, H], F32, kind="ExternalInput")
    b_enc = nc.dram_tensor("b_enc", [128, 2], F32, kind="ExternalInput")
    w_top = nc.dram_tensor("w_top", [K, 2, 128, H], F32, kind="ExternalInput")
    w_bot = nc.dram_tensor("w_bot", [K, 2, 128, H], F32, kind="ExternalInput")
    b_h = nc.dram_tensor("b_h", [128, K * 2], F32, kind="ExternalInput")
    w_dec = nc.dram_tensor("w_dec", [2, 128, DOUT], F32, kind="ExternalInput")
    b_dec = nc.dram_tensor("b_dec", [DOUT, 1], F32, kind="ExternalInput")
    sel = nc.dram_tensor("sel", [ST_EX, ST], F32, kind="ExternalInput")
    y = nc.dram_tensor("y", [DOUT, tok], F32, kind="ExternalOutput")

    with tile.TileContext(nc) as tc:
        with (
            tc.tile_pool(name="wpool", bufs=1) as wpool,
            tc.tile_pool(name="xpool", bufs=2) as xpool,
            tc.tile_pool(name="hpool", bufs=2) as hpool,
            tc.tile_pool(name="opool", bufs=2) as opool,
            tc.tile_pool(name="cpool", bufs=2) as cpool,
            tc.tile_pool(name="psmm", bufs=4, space=bass.MemorySpace.PSUM) as psmm,
            tc.tile_pool(name="psdec", bufs=2, space=bass.MemorySpace.PSUM) as psdec,
            tc.tile_pool(name="pscw", bufs=2, space=bass.MemorySpace.PSUM) as pscw,
        ):
            # --- weights: load once, stay resident ---
            wenc_sb = wpool.tile([DIN, H], F32R)
            nc.gpsimd.dma_start(wenc_sb[:], w_enc[:])
            benc_sb = wpool.tile([128, 2], F32)
            nc.sync.dma_start(benc_sb[:], b_enc[:])
            wtop_sb = wpool.tile([128, K * 2 * H], F32R)
            wbot_sb = wpool.tile([128, K * 2 * H], F32R)
            for k in range(K):
                for kc in range(2):
                    off = (k * 2 + kc) * H
                    nc.gpsimd.dma_start(wtop_sb[:, off : off + H], w_top[k, kc])
                    nc.gpsimd.dma_start(wbot_sb[:, off : off + H], w_bot[k, kc])
            bh_sb = wpool.tile([128, K * 2], F32)
            nc.sync.dma_start(bh_sb[:], b_h[:])
            wdec_sb = wpool.tile([128, 2 * DOUT], F32R)
            for kc in range(2):
                nc.gpsimd.dma_start(wdec_sb[:, kc * DOUT : (kc + 1) * DOUT], w_dec[kc])
            bdec_sb = wpool.tile([DOUT, 1], F32)
            nc.sync.dma_start(bdec_sb[:], b_dec[:])
            sel_sb = wpool.tile([ST_EX, ST], F32R)
            nc.gpsimd.dma_start(sel_sb[:], sel[:])

            for s in range(n_supertiles):
                t0 = s * ST
                xt = xpool.tile([DIN, ST], F32R, tag="xt")
                nc.gpsimd.dma_start(xt[:], xT[:, t0 : t0 + ST])
                hA = [hpool.tile([128, ST], F32R, tag=f"hA{m}", name=f"hA{m}_{s}") for m in range(2)]
                hB = [hpool.tile([128, ST], F32R, tag=f"hB{m}", name=f"hB{m}_{s}") for m in range(2)]

                # encoder: h = tanh(W_enc.T @ xT + b_enc)
                for m in range(2):
                    for n in range(NSUB):
                        ps = psmm.tile([128, SUB], F32, tag="ps")
                        nc.tensor.matmul(
                            ps[:],
                            wenc_sb[:, m * 128 : (m + 1) * 128],
                            xt[:, n * SUB : (n + 1) * SUB],
                            start=True,
                            stop=True,
                        )
                        nc.scalar.activation(
                            hA[m][:, n * SUB : (n + 1) * SUB],
                            ps[:],
                            Tanh,
                            bias=benc_sb[:, m : m + 1],
                        )

                hcur, hnxt = hA, hB
                for k in range(K):
                    # per-example agent sum (1/50 pre-folded into W_bot)
                    c_t = cpool.tile([128, 2, ST_EX], F32R, tag="c")
                    for kc in range(2):
                        for n in range(NSUB):
                            seg = (
                                hcur[kc][:, n * SUB : (n + 1) * SUB]
                                .bitcast(F32)
                                .rearrange("p (b a) -> p b a", a=A)
                            )
                            with nc.allow_low_precision(
                                reason="f32r out rounding; accumulation is fp32"
                            ):
                                nc.vector.reduce_sum(
                                    c_t[:, kc, n * SUB_EX : (n + 1) * SUB_EX],
                                    seg,
                                    axis=mybir.AxisListType.X,
                                )
                    # cwT[ex, feat] = c.T @ W_bot  (c is already [feat, ex] = lhsT)
                    pcw = pscw.tile([ST_EX, H], F32, tag="pcw")
                    for kc in range(2):
                        off = (k * 2 + kc) * H
                        nc.tensor.matmul(
                            pcw[:],
                            c_t[:, kc, :],
                            wbot_sb[:, off : off + H],
                            start=(kc == 0),
                            stop=(kc == 1),
                        )
                    cwT_sb = cpool.tile([ST_EX, H], F32R, tag="cwT")
                    nc.vector.tensor_copy(cwT_sb[:], pcw[:])
                    # h' = tanh(W_top.T @ h + cw(bcast over agents via K=8
                    # selector matmul) + b_h)
                    for m in range(2):
                        for n in range(NSUB):
                            ps = psmm.tile([128, SUB], F32, tag="ps")
                            for kc in range(2):
                                off = (k * 2 + kc) * H + m * 128
                                nc.tensor.matmul(
                                    ps[:],
                                    wtop_sb[:, off : off + 128],
                                    hcur[kc][:, n * SUB : (n + 1) * SUB],
                                    start=(kc == 0),
                                    stop=False,
                                )
                            nc.tensor.matmul(
                                ps[:],
                                cwT_sb[:, m * 128 : (m + 1) * 128],
                                sel_sb[:, n * SUB : (n + 1) * SUB],
                                start=False,
                                stop=True,
                            )
                            nc.scalar.activation(
                                hnxt[m][:, n * SUB : (n + 1) * SUB],
                                ps[:],
                                Tanh,
                                bias=bh_sb[:, k * 2 + m : k * 2 + m + 1],
                            )
                    hcur, hnxt = hnxt, hcur

                # decoder: y = W_dec.T @ h + b_dec
                out_t = opool.tile([DOUT, ST], F32, tag="out")
                for n in range(NSUB):
                    pd = psdec.tile([DOUT, SUB], F32, tag="pd")
                    for kc in range(2):
                        nc.tensor.matmul(
                            pd[:],
                            wdec_sb[:, kc * DOUT : (kc + 1) * DOUT],
                            hcur[kc][:, n * SUB : (n + 1) * SUB],
                            start=(kc == 0),
                            stop=(kc == 1),
                        )
                    nc.vector.tensor_scalar_add(
                        out_t[:, n * SUB : (n + 1) * SUB], pd[:], bdec_sb[:, 0:1]
                    )
                nc.sync.dma_start(y[:, t0 : t0 + ST], out_t[:])

    nc.compile()
    return nc


def host_inputs(x, W_enc, b_enc, W_h, b_h, W_dec, b_dec, n_cores=N_CORES, bs=BS):
    """Shard x over cores (pre-transposed to [DIN, tok]); replicate weights."""
    x = np.asarray(x, np.float32)
    common = {
        "w_enc": np.ascontiguousarray(np.asarray(W_enc, np.float32)),
        "b_enc": np.ascontiguousarray(
            np.asarray(b_enc, np.float32).reshape(2, 128).T
        ),
        "w_top": np.ascontiguousarray(
            np.asarray(W_h, np.float32)[:, :H, :].reshape(K, 2, 128, H)
        ),
        "w_bot": np.ascontiguousarray(
            (np.asarray(W_h, np.float32)[:, H:, :] / A).reshape(K, 2, 128, H)
        ),
        "b_h": np.ascontiguousarray(
            np.asarray(b_h, np.float32).reshape(K, 2, 128).transpose(2, 0, 1).reshape(128, K * 2)
        ),
        "w_dec": np.ascontiguousarray(
            np.asarray(W_dec, np.float32).reshape(2, 128, DOUT)
        ),
        "b_dec": np.ascontiguousarray(np.asarray(b_dec, np.float32).reshape(DOUT, 1)),
        "sel": np.ascontiguousarray(
            np.repeat(np.eye(ST_EX, dtype=np.float32), A, axis=1)
        ),
    }
    in_maps = []
    for i in range(n_cores):
        shard = x[i * bs : (i + 1) * bs].reshape(bs * A, DIN)
        in_maps.append({**common, "xT": np.ascontiguousarray(shard.T)})
    return in_maps


_NC_CACHE = None


def _get_nc():
    global _NC_CACHE
    if _NC_CACHE is None:
        _NC_CACHE = build_nc()
    return _NC_CACHE


def kernel(x, W_enc, b_enc, W_h, b_h, W_dec, b_dec, _run_kwargs=None):
    in_maps = host_inputs(x, W_enc, b_enc, W_h, b_h, W_dec, b_dec)
    nc = _get_nc()
    res = run_bass_kernel_spmd(nc, in_maps, list(range(N_CORES)), **(_run_kwargs or {}))
    outs = [
        res.results[i]["y"].T.reshape(BS, A, DOUT).astype(np.float32)
        for i in range(N_CORES)
    ]
    full = np.concatenate(outs, axis=0)
    if _run_kwargs:
        kernel.last_results = res
    return full
